# revision 4
# baseline (speedup 1.0000x reference)
"""GCN layer (gather -> x@W -> normalized scatter-add -> bias -> PReLU) on 8 trn2 cores.

Strategy (node sharding, per spec hint):
  - 100000 nodes padded to 100352 = 8 * 12544; core c owns node range [c*12544, (c+1)*12544).
  - Phase 1: each core computes hs = dinv * (x_own @ W) for its nodes (fp32 matmul on PE,
    x tiles transposed on PE), stores as bf16.
  - Phase 2: AllGather -> every core holds full hs [100352, 128] bf16.
  - Phase 3: edges sorted by destination; destinations processed in 128-node windows
    (PSUM [128 dst, 128 feat]); per 128-edge block a one-hot S matrix [edge, dst] is built
    on DVE (iota == reldst) and PE computes psum += S^T @ G where G = gathered hs rows
    (dma_gather, int16 idx => sources split in 4 quarters of 25088 rows).
    Self-loops are folded as one identity matmul per window on contiguous hs rows.
    Epilogue: out = prelu(dinv_dst * psum + b) with per-feature alpha.
"""
import sys
sys.path.insert(0, '/opt/trn_rl_repo')

import numpy as np
import ml_dtypes

N = 100000
NCORES = 8
SH = 12544                 # nodes per core
NP = NCORES * SH           # 100352 padded nodes
H = 128                    # output features
KIN = 256                  # input features
WIN = 128                  # dst window size
NW = SH // WIN             # 98 windows per core
WG = 8                     # windows per PSUM group
NG = (NW + WG - 1) // WG   # 13 groups
NQ = 4                     # source quarters
QROWS = NP // NQ           # 25088 rows per quarter (int16-safe)

bf16 = ml_dtypes.bfloat16


def _preprocess(edge_index):
    """Sort/pad edges into the common (group, quarter, window) block schedule.

    Returns (sched, per_core) where sched is shared trace-time metadata and
    per_core has the padded idx16 / reldst streams per core.
    """
    src = np.asarray(edge_index[0]).astype(np.int64)
    dst = np.asarray(edge_index[1]).astype(np.int64)
    E = src.shape[0]

    deg = (np.bincount(dst, minlength=N) + 1).astype(np.float32)
    dinv = (1.0 / np.sqrt(deg)).astype(np.float32)
    dinv_np = np.ones(NP, np.float32)
    dinv_np[:N] = dinv

    core = dst // SH
    w_in_core = (dst % SH) // WIN            # 0..NW-1
    g = w_in_core // WG
    wi = w_in_core % WG
    q = src // QROWS

    # per-edge schedule bin: (core, g, q, wi)
    key = ((core * NG + g) * NQ + q) * WG + wi
    nbins_pc = NG * NQ * WG
    order = np.argsort(key, kind='stable')
    s_src = src[order]
    s_dst = dst[order]
    s_key = key[order]

    cnt_all = np.bincount(key, minlength=NCORES * nbins_pc)
    bin_start = np.concatenate([[0], np.cumsum(cnt_all)])[:-1]
    rank = np.arange(E, dtype=np.int64) - bin_start[s_key]

    cnt = cnt_all.reshape(NCORES, NG, NQ, WG)
    nblk = np.ceil(cnt.max(axis=0) / WIN).astype(np.int64)   # [NG, NQ, WG] common
    pad_sizes = (nblk * WIN).reshape(-1)                     # order (g, q, wi)
    offs = np.concatenate([[0], np.cumsum(pad_sizes)])
    TOT = int(offs[-1])
    NBLK = TOT // WIN

    # per-edge padded position (within its core's stream)
    bin_in_core = s_key % nbins_pc
    pos = offs[bin_in_core] + rank

    win_base = (core * SH + (g * NG * 0 + w_in_core) * WIN)  # c*SH + w*WIN
    win_base = core * SH + w_in_core * WIN
    s_win_base = win_base[order]
    s_q = q[order]
    s_core = core[order]

    per_core = []
    for c in range(NCORES):
        m = s_core == c
        idxq = np.zeros(TOT, np.int16)
        rels = np.full(TOT, -1.0, np.float32)
        p_c = pos[m]
        idxq[p_c] = (s_src[m] - s_q[m] * QROWS).astype(np.int16)
        rels[p_c] = (s_dst[m] - s_win_base[m]).astype(np.float32)
        # idx16 wrap layout: [16, TOT/16] with element i at [i%16, i//16]; replicate x8
        idx16 = np.tile(np.ascontiguousarray(idxq.reshape(TOT // 16, 16).T), (8, 1))
        relm = np.ascontiguousarray(rels.reshape(NBLK, WIN).T)   # [128, NBLK]
        dinv_own = np.ascontiguousarray(
            dinv_np[c * SH:(c + 1) * SH].reshape(NW, WIN).T)     # [128, NW]
        per_core.append(dict(idx16=idx16, reldst=relm, dinv=dinv_own))

    # static schedule: per (g, q): idx stream offset, n idx, list of (block_col, window)
    calls = []          # (g, q, off_idx, nidx, [(Bcol, w), ...])
    Bcol = 0
    last_block_of_win = {}   # w -> Bcol of its final edge-block (or None)
    for gg in range(NG):
        for qq in range(NQ):
            blocks = []
            off_idx = None
            for wii in range(WG):
                nb = int(nblk[gg, qq, wii])
                if nb == 0:
                    continue
                w = gg * WG + wii
                bin_i = (gg * NQ + qq) * WG + wii
                if off_idx is None:
                    off_idx = int(offs[bin_i])
                for k in range(nb):
                    blocks.append((Bcol, w))
                    last_block_of_win[w] = Bcol
                    Bcol += 1
            if blocks:
                calls.append((gg, qq, off_idx, len(blocks) * WIN, blocks))
    assert Bcol == NBLK
    sched = dict(calls=calls, last_block=last_block_of_win, NBLK=NBLK, TOT=TOT)
    return sched, per_core, dinv_np


def _build(sched):
    import os
    EN_GATHER = os.environ.get("K_NO_GATHER", "") == ""
    EN_COLL = os.environ.get("K_NO_COLL", "") == ""
    from concourse import bass, bacc, tile, mybir

    nc = bacc.Bacc("TRN2", target_bir_lowering=False, debug=False,
                   enable_asserts=True, num_devices=NCORES)

    x_d = nc.dram_tensor("x_own", [SH, KIN], mybir.dt.float32, kind="ExternalInput")
    w_d = nc.dram_tensor("w_mat", [KIN, H], mybir.dt.float32, kind="ExternalInput")
    b_d = nc.dram_tensor("b_vec", [H], mybir.dt.float32, kind="ExternalInput")
    a_d = nc.dram_tensor("a_vec", [H], mybir.dt.float32, kind="ExternalInput")
    dinv_d = nc.dram_tensor("dinv_own", [128, NW], mybir.dt.float32, kind="ExternalInput")
    idx_d = nc.dram_tensor("idx16", [128, sched["TOT"] // 16], mybir.dt.int16, kind="ExternalInput")
    rel_d = nc.dram_tensor("reldst", [128, sched["NBLK"]], mybir.dt.float32, kind="ExternalInput")

    out_d = nc.dram_tensor("out_own", [SH, H], mybir.dt.float32, kind="ExternalOutput")

    hs_own = nc.dram_tensor("hs_own", [SH, H], mybir.dt.bfloat16)
    hs_full = nc.dram_tensor("hs_full", [NP, H], mybir.dt.bfloat16, addr_space="Shared")

    calls = sched["calls"]
    last_block = sched["last_block"]
    max_call_blk = max(len(cb[4]) for cb in calls)
    nblk_of_group = [sum(len(cb[4]) for cb in calls if cb[0] == gg) for gg in range(NG)]
    first_col_of_group = [min([cb[4][0][0] for cb in calls if cb[0] == gg] or [0])
                          for gg in range(NG)]

    with tile.TileContext(nc) as tc:
        with tc.tile_pool(name="consts", bufs=1) as cp, tc.tile_pool(name="sb", bufs=3) as sb:
            # ---------------- constants ----------------
            iota_i = cp.tile([128, 128], mybir.dt.int32)
            nc.gpsimd.iota(iota_i[:], pattern=[[1, 128]], base=0, channel_multiplier=0)
            iota_f = cp.tile([128, 128], mybir.dt.float32)
            nc.vector.tensor_copy(iota_f[:], iota_i[:])

            ident_f = cp.tile([128, 128], mybir.dt.float32)
            from concourse.masks import make_identity
            make_identity(nc, ident_f[:])
            ident_b = cp.tile([128, 128], mybir.dt.bfloat16)
            nc.vector.tensor_copy(ident_b[:], ident_f[:])

            w0 = cp.tile([128, H], mybir.dt.float32)
            w1 = cp.tile([128, H], mybir.dt.float32)
            nc.sync.dma_start(w0[:], w_d[0:128, :])
            nc.sync.dma_start(w1[:], w_d[128:256, :])

            dinv_sb = cp.tile([128, NW], mybir.dt.float32)
            nc.sync.dma_start(dinv_sb[:], dinv_d[:, :])

            ones1 = cp.tile([1, H], mybir.dt.float32)
            nc.vector.memset(ones1[:], 1.0)
            bvec = cp.tile([1, H], mybir.dt.float32)
            nc.sync.dma_start(bvec[:], b_d[None, :])
            avec = cp.tile([1, H], mybir.dt.float32)
            nc.sync.dma_start(avec[:], a_d[None, :])

            b128 = cp.tile([128, H], mybir.dt.float32)
            a128 = cp.tile([128, H], mybir.dt.float32)

            with tc.tile_pool(name="psum1", bufs=2, space="PSUM") as pp1:
                bc_ps = pp1.tile([128, H], mybir.dt.float32, space="PSUM", tag="bc")
                nc.tensor.matmul(out=bc_ps[:], lhsT=ones1[:], rhs=bvec[:], start=True, stop=True)
                nc.vector.tensor_copy(b128[:], bc_ps[:])
                ac_ps = pp1.tile([128, H], mybir.dt.float32, space="PSUM", tag="bc")
                nc.tensor.matmul(out=ac_ps[:], lhsT=ones1[:], rhs=avec[:], start=True, stop=True)
                nc.vector.tensor_copy(a128[:], ac_ps[:])

                # ---------------- phase 1: hs = dinv * (x @ W) ----------------
                for i in range(NW):
                    x_t = sb.tile([128, KIN], mybir.dt.float32, tag="x_t")
                    nc.sync.dma_start(x_t[:], x_d[i * 128:(i + 1) * 128, :])
                    h_ps = pp1.tile([128, H], mybir.dt.float32, space="PSUM", tag="h_ps")
                    for kk in range(2):
                        xt_ps = pp1.tile([128, 128], mybir.dt.float32, space="PSUM", tag="xt_ps")
                        nc.tensor.transpose(xt_ps[:], x_t[:, kk * 128:(kk + 1) * 128], ident_f[:])
                        xt_sb = sb.tile([128, 128], mybir.dt.float32, tag="xt_sb")
                        nc.scalar.activation(xt_sb[:], xt_ps[:], mybir.ActivationFunctionType.Copy)
                        nc.tensor.matmul(out=h_ps[:], lhsT=xt_sb[:], rhs=(w0 if kk == 0 else w1)[:],
                                         start=(kk == 0), stop=(kk == 1))
                    hs_t = sb.tile([128, H], mybir.dt.bfloat16, tag="hs_t")
                    nc.vector.tensor_scalar(out=hs_t[:], in0=h_ps[:],
                                            scalar1=dinv_sb[:, i:i + 1], scalar2=None,
                                            op0=mybir.AluOpType.mult)
                    nc.sync.dma_start(hs_own[i * 128:(i + 1) * 128, :], hs_t[:])

            # ---------------- phase 2: AllGather ----------------
            if EN_COLL:
                nc.gpsimd.collective_compute(
                    "AllGather", mybir.AluOpType.bypass,
                    replica_groups=[list(range(NCORES))],
                    ins=[hs_own.ap().opt()],
                    outs=[hs_full.ap().opt()],
                )

            # ---------------- phase 3: scatter windows ----------------
            with tc.tile_pool(name="psum3", bufs=WG, space="PSUM") as pp3:
                for gg in range(NG):
                    wlo = gg * WG
                    whi = min(wlo + WG, NW)
                    pw = {}
                    for w in range(wlo, whi):
                        pw[w] = pp3.tile([128, H], mybir.dt.float32, space="PSUM",
                                         tag="pw", name=f"pw{w}")
                        hs_self = sb.tile([128, H], mybir.dt.bfloat16, tag="hs_self")
                        nc.sync.dma_start(hs_self[:], hs_own[w * 128:(w + 1) * 128, :])
                        w_done = (w not in last_block) or (not EN_GATHER)
                        nc.tensor.matmul(out=pw[w][:], lhsT=ident_b[:], rhs=hs_self[:],
                                         start=True, stop=w_done)

                    if nblk_of_group[gg]:
                        rd_sb = sb.tile([128, max(nblk_of_group)], mybir.dt.float32, tag="rd")
                        c0 = first_col_of_group[gg]
                        nc.sync.dma_start(rd_sb[:, 0:nblk_of_group[gg]],
                                          rel_d[:, c0:c0 + nblk_of_group[gg]])

                    for (g_c, qq, off_idx, nidx, blocks) in calls:
                        if g_c != gg or not EN_GATHER:
                            continue
                        idx_sb = sb.tile([128, max_call_blk * 8], mybir.dt.int16, tag="idx")
                        nc.sync.dma_start(idx_sb[:, 0:nidx // 16],
                                          idx_d[:, off_idx // 16: (off_idx + nidx) // 16])
                        g_t = sb.tile([128, max_call_blk, H], mybir.dt.bfloat16, tag="g_t")
                        nc.gpsimd.dma_gather(
                            g_t[:, 0:nidx // 128, :], hs_full[qq * QROWS:(qq + 1) * QROWS, :],
                            idx_sb[:, 0:nidx // 16], nidx, nidx, H,
                            single_packet=False)
                        for (bcol, w) in blocks:
                            s_t = sb.tile([128, 128], mybir.dt.bfloat16, tag="s_t")
                            lc = bcol - first_col_of_group[gg]
                            nc.vector.tensor_scalar(
                                out=s_t[:], in0=iota_f[:],
                                scalar1=rd_sb[:, lc:lc + 1], scalar2=None,
                                op0=mybir.AluOpType.is_equal)
                            slot = (bcol - blocks[0][0])
                            nc.tensor.matmul(out=pw[w][:], lhsT=s_t[:], rhs=g_t[:, slot, :],
                                             start=False, stop=(last_block.get(w) == bcol))

                    # epilogue per window
                    for w in range(wlo, whi):
                        u = sb.tile([128, H], mybir.dt.float32, tag="u")
                        nc.scalar.activation(u[:], pw[w][:], mybir.ActivationFunctionType.Copy,
                                             scale=dinv_sb[:, w:w + 1])
                        u2 = sb.tile([128, H], mybir.dt.float32, tag="u2")
                        nc.vector.tensor_tensor(out=u2[:], in0=u[:], in1=b128[:],
                                                op=mybir.AluOpType.add)
                        r2 = sb.tile([128, H], mybir.dt.float32, tag="r2")
                        nc.scalar.activation(r2[:], u2[:], mybir.ActivationFunctionType.Relu,
                                             scale=-1.0)
                        m = sb.tile([128, H], mybir.dt.float32, tag="m")
                        nc.vector.tensor_tensor(out=m[:], in0=r2[:], in1=a128[:],
                                                op=mybir.AluOpType.mult)
                        r1 = sb.tile([128, H], mybir.dt.float32, tag="r1")
                        nc.scalar.activation(r1[:], u2[:], mybir.ActivationFunctionType.Relu)
                        o = sb.tile([128, H], mybir.dt.float32, tag="o")
                        nc.vector.tensor_tensor(out=o[:], in0=r1[:], in1=m[:],
                                                op=mybir.AluOpType.subtract)
                        nc.sync.dma_start(out_d[w * 128:(w + 1) * 128, :], o[:])

    nc.compile()
    return nc


_CACHE = {}


def _get_nc_and_sched(edge_index):
    sched, per_core, dinv_np = _preprocess(edge_index)
    nc = _build(sched)
    return nc, sched, per_core


def kernel(x, edge_index, W, b, alpha):
    from concourse.bass_utils import run_bass_kernel_spmd

    x = np.asarray(x, dtype=np.float32)
    W = np.asarray(W, dtype=np.float32)
    b = np.asarray(b, dtype=np.float32)
    alpha = np.asarray(alpha, dtype=np.float32)

    nc, sched, per_core = _get_nc_and_sched(edge_index)

    x_pad = np.zeros((NP, KIN), np.float32)
    x_pad[:N] = x

    in_maps = []
    for c in range(NCORES):
        in_maps.append({
            "x_own": np.ascontiguousarray(x_pad[c * SH:(c + 1) * SH]),
            "w_mat": W, "b_vec": b, "a_vec": alpha,
            "dinv_own": per_core[c]["dinv"],
            "idx16": per_core[c]["idx16"],
            "reldst": per_core[c]["reldst"],
        })

    res = run_bass_kernel_spmd(nc, in_maps, core_ids=list(range(NCORES)))
    out = np.concatenate([res.results[c]["out_own"] for c in range(NCORES)], axis=0)
    return np.ascontiguousarray(out[:N])
